# revision 17
# baseline (speedup 1.0000x reference)
"""ExpertScatter TRN2 kernel — direct DMA scatter-add, parity-split outputs.

reference semantics:
    X = einsum('bekj,eji->beki', Y, W)          # per-head projection
    out[b] = zeros([T, I]); out[b, Ind[b,e,k]] += X[b,e,k]

Strategy (data-parallel over batch, 1 batch per NeuronCore):
  Per head e, project Yt chunks against W[e] on the PE (fp16 operands,
  fp32 PSUM), copy each [128, 1024] X chunk to SBUF (fp16), and
  scatter-add the rows straight into HBM output accumulators with
  gpsimd.dma_scatter_add (out[idx_i] += x_row_i).

  Duplicate-index hazard: descriptors of ONE scatter call race on HBM
  read-modify-write (probed: duplicates closer than ~256 positions lose
  adds). Fix: the host pre-sums rows of the same (head, slot) — exact,
  since they share W[e] — so each call has unique indices. Cross-head
  duplicates are safe: calls targeting the same tensor are
  WAW-serialized by the tile framework (next call's transfer waits the
  previous DMA-completion semaphore).

  The WAW chain leaves a ~3us bubble between consecutive scatter calls
  (sem propagation + Q7 descriptor-gen + DGE trigger delay). To fill
  those bubbles, the output is split by slot parity into TWO separate
  DRAM tensors (out_even = slots 0,2,..., out_odd = slots 1,3,...) with
  independent WAW chains; their transfers interleave on the shared DMA
  engines, and the host re-interleaves rows at the end (free).

  Zero-payload pad tokens target a dedicated DUMP ROW (row T/2) so
  their RMWs can never race a real slot's add, and per-call num_idxs is
  capped at the data's max real count (shared across cores, SPMD-safe).

  Total per-core DMA: Y 4MB + W 4MB + zero-init 8MB + scatter ~30MB
  = ~46MB at ~360GB/s model bandwidth (~98% DMA-device utilization),
  vs ~80MB for the sort+stage+gather pipeline (286us -> 139us).
"""

import os

import numpy as np

import concourse.bacc as bacc
import concourse.mybir as mybir
import concourse.tile as tile
from concourse.bass_utils import run_bass_kernel_spmd

# Problem constants (hardcoded per harness contract).
B = 8
HEADS = 16
K = 1024
HEAD_DIM = 128
OUT_DIM = 1024
T_SLOTS = 4096
NCORES = 8

PP = 512                 # padded rows per (head, parity); 487 on seed-0 data

F32 = mybir.dt.float32
F16 = mybir.dt.float16
I16 = mybir.dt.int16

XBUFS = int(os.environ.get("ES_XBUFS", "3"))
PBUFS = int(os.environ.get("ES_PBUFS", "3"))
YBUFS = int(os.environ.get("ES_YBUFS", "4"))

_cache = {}


def _build_program(pp, caps):
    ng = (2 * pp) // 128                  # X groups per head
    gpar = ng // 2                        # groups per parity
    kh = 2 * pp                           # padded rows per head
    # SBUF pressure guard for the (never-seen) large-pp fallback path.
    xbufs = XBUFS if pp <= 512 else 2
    ybufs = YBUFS if pp <= 512 else 2
    nc = bacc.Bacc("TRN2", target_bir_lowering=False, debug=False,
                   num_devices=NCORES, num_swdge_queues=2,
                   dynamic_dma_scratch_size=131072)

    yt = nc.dram_tensor("yt", [HEAD_DIM, HEADS * kh], F16,
                        kind="ExternalInput").ap()
    w = nc.dram_tensor("w", [HEAD_DIM, HEADS * OUT_DIM], F16,
                       kind="ExternalInput").ap()
    idx = nc.dram_tensor("idx", [128, HEADS * kh // 16], I16,
                         kind="ExternalInput").ap()
    # One extra "dump" row per tensor: zero-payload pad tokens target it
    # so their HBM read-modify-writes can never race a real slot's add.
    outs = [nc.dram_tensor(nm, [T_SLOTS // 2 + 1, OUT_DIM], F16,
                           kind="ExternalOutput").ap()
            for nm in ("out_even", "out_odd")]

    with tile.TileContext(nc) as tc:
        with (
            tc.tile_pool(name="const", bufs=1) as cpool,
            tc.tile_pool(name="yhead", bufs=ybufs) as ypool,
            tc.tile_pool(name="xblk", bufs=xbufs) as xpool,
            tc.tile_pool(name="psum", bufs=PBUFS, space="PSUM") as pspool,
        ):
            z = cpool.tile([128, OUT_DIM], F16, tag="z")
            nc.vector.memset(z[:], 0.0)
            idx_sb = cpool.tile([128, HEADS * kh // 16], I16, tag="idx")
            nc.sync.dma_start(out=idx_sb[:], in_=idx[:])
            w_sb = cpool.tile([128, HEADS * OUT_DIM], F16, tag="w")

            def load_w(e):
                nc.sync.dma_start(
                    out=w_sb[:, e * OUT_DIM:(e + 1) * OUT_DIM],
                    in_=w[:, e * OUT_DIM:(e + 1) * OUT_DIM])

            yts = {}

            def load_yt(e):
                yt_e = ypool.tile([128, kh], F16, tag="yt")
                nc.sync.dma_start(out=yt_e[:],
                                  in_=yt[:, e * kh:(e + 1) * kh])
                yts[e] = yt_e

            # Early loads so heads 0-3 compute overlaps the zero-init.
            for e in range(4):
                load_w(e)
                load_yt(e)
            # Zero-init the accumulators: even chain issued from SP (early),
            # odd chain from ACT; both ahead of their first scatters.
            for t in range(T_SLOTS // 256):
                nc.sync.dma_start(out=outs[0][t * 128:(t + 1) * 128, :],
                                  in_=z[:])
            for t in range(T_SLOTS // 256):
                nc.scalar.dma_start(out=outs[1][t * 128:(t + 1) * 128, :],
                                    in_=z[:])
            for e in range(4, HEADS):
                load_w(e)

            for e in range(HEADS):
                xb = xpool.tile([128, ng, OUT_DIM], F16, tag="xb")
                if e not in yts:
                    load_yt(e)
                yt_e = yts[e]
                for rc in range(ng):
                    px = pspool.tile([128, OUT_DIM], F32, tag="px")
                    for h in range(2):
                        nc.tensor.matmul(
                            out=px[:, h * 512:(h + 1) * 512],
                            lhsT=yt_e[:, rc * 128:(rc + 1) * 128],
                            rhs=w_sb[:, e * OUT_DIM + h * 512:
                                     e * OUT_DIM + (h + 1) * 512],
                            start=True, stop=True,
                        )
                    if rc % 2 == 0:
                        nc.vector.tensor_copy(out=xb[:, rc, :], in_=px[:])
                    else:
                        nc.scalar.copy(out=xb[:, rc, :], in_=px[:])
                for par in range(2):
                    cap = caps[e * 2 + par]
                    gcap = -(-cap // 128)      # in_ap groups = roundup128(cap)
                    nc.gpsimd.dma_scatter_add(
                        outs[par][:],
                        xb[:, par * gpar:par * gpar + gcap, :],
                        idx_sb[:, (e * 2 + par) * (pp // 16):
                               (e * 2 + par) * (pp // 16) + cap // 16],
                        cap,
                        cap,
                        OUT_DIM,
                        queue_num=par,
                    )

    nc.compile()
    return nc


def _get_program(pp, caps):
    key = (pp, caps)
    if key not in _cache:
        _cache[key] = _build_program(pp, caps)
    return _cache[key]


def _dedup_head(Yh, indh):
    """Pre-sum rows sharing a slot (exact: same W)."""
    order = np.argsort(indh, kind="stable")
    sind = indh[order]
    starts = np.concatenate(([0], np.nonzero(np.diff(sind))[0] + 1))
    sums = np.add.reduceat(Yh[order].astype(np.float32), starts, axis=0)
    return sums, sind[starts]


def kernel(Y, Ind, T, W):
    Y = np.asarray(Y, dtype=np.float32)
    Ind = np.asarray(Ind)
    W = np.asarray(W, dtype=np.float32)
    assert int(T) == T_SLOTS and Y.shape == (B, HEADS, K, HEAD_DIM)

    w_in = np.ascontiguousarray(
        W.transpose(1, 0, 2).reshape(HEAD_DIM, HEADS * OUT_DIM)
    ).astype(np.float16)

    # Host prep: dedup per head, split by slot parity.
    per_core = []
    cnts = np.zeros((B, HEADS * 2), dtype=np.int64)
    for b in range(B):
        heads = []
        for e in range(HEADS):
            sums, slots = _dedup_head(Y[b, e], Ind[b, e].astype(np.int64))
            parts = []
            for par in range(2):
                m = (slots % 2) == par
                parts.append((sums[m], (slots[m] // 2).astype(np.int16)))
                cnts[b, e * 2 + par] = int(m.sum())
            heads.append(parts)
        per_core.append(heads)
    maxcnt = int(cnts.max())
    pp = max(PP, -(-maxcnt // 128) * 128)
    kh = 2 * pp
    # Per-call num_idxs cap: shared across cores (SPMD), tight per call.
    caps = tuple(int(-(-int(c) // 16) * 16) or 16
                 for c in cnts.max(axis=0))

    in_maps = []
    for b in range(B):
        ytb = np.zeros((HEAD_DIM, HEADS * kh), dtype=np.float16)
        idx_cols = []
        for e in range(HEADS):
            for par in range(2):
                rows, slots = per_core[b][e][par]
                cnt = len(slots)
                base = e * kh + par * pp
                ytb[:, base:base + cnt] = rows.T.astype(np.float16)
                idxp = np.full(pp, T_SLOTS // 2, dtype=np.int16)
                idxp[:cnt] = slots
                idx_cols.append(np.tile(idxp.reshape(pp // 16, 16).T, (8, 1)))
        idx_in = np.ascontiguousarray(np.concatenate(idx_cols, axis=1),
                                      dtype=np.int16)
        in_maps.append({"yt": ytb, "w": w_in, "idx": idx_in})

    nc = _get_program(pp, caps)

    # The first execution of a freshly compiled NEFF occasionally wedges a
    # core (NRT_EXEC_UNIT_UNRECOVERABLE); a retry on a fresh execute has
    # been observed to recover.
    last_exc = None
    for attempt in range(3):
        try:
            res = run_bass_kernel_spmd(
                nc, in_maps, core_ids=list(range(NCORES)),
                trace=os.environ.get("ES_TRACE", "0") == "1",
            )
            break
        except Exception as exc:  # noqa: BLE001 - device flake, retry
            last_exc = exc
            import time as _time
            _time.sleep(2.0)
    else:
        raise last_exc
    kernel.last_results = res
    out = np.empty((B, T_SLOTS, OUT_DIM), dtype=np.float32)
    for b in range(B):
        out[b, 0::2] = res.results[b]["out_even"][:T_SLOTS // 2].astype(np.float32)
        out[b, 1::2] = res.results[b]["out_odd"][:T_SLOTS // 2].astype(np.float32)
    return out


# revision 18
# speedup vs baseline: 1.0203x; 1.0203x over previous
"""ExpertScatter TRN2 kernel — direct DMA scatter-add, parity-split outputs.

reference semantics:
    X = einsum('bekj,eji->beki', Y, W)          # per-head projection
    out[b] = zeros([T, I]); out[b, Ind[b,e,k]] += X[b,e,k]

Strategy (data-parallel over batch, 1 batch per NeuronCore):
  Per head e, project Yt chunks against W[e] on the PE (fp16 operands,
  fp32 PSUM), copy each [128, 1024] X chunk to SBUF (fp16), and
  scatter-add the rows straight into HBM output accumulators with
  gpsimd.dma_scatter_add (out[idx_i] += x_row_i).

  Duplicate-index hazard: descriptors of ONE scatter call race on HBM
  read-modify-write (probed: duplicates closer than ~256 positions lose
  adds). Fix: the host pre-sums rows of the same (head, slot) — exact,
  since they share W[e] — so each call has unique indices. Cross-head
  duplicates are safe: calls targeting the same tensor are
  WAW-serialized by the tile framework (next call's transfer waits the
  previous DMA-completion semaphore).

  The WAW chain leaves a ~3us bubble between consecutive scatter calls
  (sem propagation + Q7 descriptor-gen + DGE trigger delay). To fill
  those bubbles, the output is split by slot parity into TWO separate
  DRAM tensors (out_even = slots 0,2,..., out_odd = slots 1,3,...) with
  independent WAW chains; their transfers interleave on the shared DMA
  engines, and the host re-interleaves rows at the end (free).

  Zero-payload pad tokens target a dedicated DUMP ROW (row T/2) so
  their RMWs can never race a real slot's add, and per-call num_idxs is
  capped at the data's max real count (shared across cores, SPMD-safe).

  Total per-core DMA: Y 4MB + W 4MB + zero-init 8MB + scatter ~30MB
  = ~46MB at ~360GB/s model bandwidth (~98% DMA-device utilization),
  vs ~80MB for the sort+stage+gather pipeline (286us -> 139us).
"""

import os

import numpy as np

import concourse.bacc as bacc
import concourse.mybir as mybir
import concourse.tile as tile
from concourse.bass_utils import run_bass_kernel_spmd

# Problem constants (hardcoded per harness contract).
B = 8
HEADS = 16
K = 1024
HEAD_DIM = 128
OUT_DIM = 1024
T_SLOTS = 4096
NCORES = 8

PP = 512                 # padded rows per (head, parity); 487 on seed-0 data

F32 = mybir.dt.float32
F16 = mybir.dt.float16
I16 = mybir.dt.int16

XBUFS = int(os.environ.get("ES_XBUFS", "3"))
PBUFS = int(os.environ.get("ES_PBUFS", "3"))
YBUFS = int(os.environ.get("ES_YBUFS", "4"))

_cache = {}


def _build_program(pp, caps):
    ng = (2 * pp) // 128                  # X groups per head
    gpar = ng // 2                        # groups per parity
    kh = 2 * pp                           # padded rows per head
    # SBUF pressure guard for the (never-seen) large-pp fallback path.
    xbufs = XBUFS if pp <= 512 else 2
    ybufs = YBUFS if pp <= 512 else 2
    nc = bacc.Bacc("TRN2", target_bir_lowering=False, debug=False,
                   num_devices=NCORES, num_swdge_queues=2,
                   dynamic_dma_scratch_size=131072)

    yt = nc.dram_tensor("yt", [HEAD_DIM, HEADS * kh], F16,
                        kind="ExternalInput").ap()
    w = nc.dram_tensor("w", [HEAD_DIM, HEADS * OUT_DIM], F16,
                       kind="ExternalInput").ap()
    idx = nc.dram_tensor("idx", [128, HEADS * kh // 16], I16,
                         kind="ExternalInput").ap()
    # One extra "dump" row per tensor: zero-payload pad tokens target it
    # so their HBM read-modify-writes can never race a real slot's add.
    outs = [nc.dram_tensor(nm, [T_SLOTS // 2 + 1, OUT_DIM], F16,
                           kind="ExternalOutput").ap()
            for nm in ("out_even", "out_odd")]

    with tile.TileContext(nc) as tc:
        with (
            tc.tile_pool(name="const", bufs=1) as cpool,
            tc.tile_pool(name="yhead", bufs=ybufs) as ypool,
            tc.tile_pool(name="xblk", bufs=xbufs) as xpool,
            tc.tile_pool(name="psum", bufs=PBUFS, space="PSUM") as pspool,
        ):
            z = cpool.tile([128, OUT_DIM], F16, tag="z")
            nc.vector.memset(z[:], 0.0)
            idx_sb = cpool.tile([128, HEADS * kh // 16], I16, tag="idx")
            nc.sync.dma_start(out=idx_sb[:], in_=idx[:])
            w_sb = cpool.tile([128, HEADS * OUT_DIM], F16, tag="w")

            def load_w(e):
                nc.sync.dma_start(
                    out=w_sb[:, e * OUT_DIM:(e + 1) * OUT_DIM],
                    in_=w[:, e * OUT_DIM:(e + 1) * OUT_DIM])

            yts = {}

            def load_yt(e):
                yt_e = ypool.tile([128, kh], F16, tag="yt")
                nc.sync.dma_start(out=yt_e[:],
                                  in_=yt[:, e * kh:(e + 1) * kh])
                yts[e] = yt_e

            # Early loads so heads 0-3 compute overlaps the zero-init.
            for e in range(4):
                load_w(e)
                load_yt(e)
            # Zero-init the accumulators: even chain issued from SP (early),
            # odd chain from ACT; both ahead of their first scatters.
            for t in range(T_SLOTS // 256):
                nc.sync.dma_start(out=outs[0][t * 128:(t + 1) * 128, :],
                                  in_=z[:])
            for t in range(T_SLOTS // 256):
                nc.scalar.dma_start(out=outs[1][t * 128:(t + 1) * 128, :],
                                    in_=z[:])
            for e in range(4, HEADS):
                load_w(e)

            for e in range(HEADS):
                xb = xpool.tile([128, ng, OUT_DIM], F16, tag="xb")
                if e not in yts:
                    load_yt(e)
                yt_e = yts[e]
                for rc in range(ng):
                    px = pspool.tile([128, OUT_DIM], F32, tag="px")
                    for h in range(2):
                        nc.tensor.matmul(
                            out=px[:, h * 512:(h + 1) * 512],
                            lhsT=yt_e[:, rc * 128:(rc + 1) * 128],
                            rhs=w_sb[:, e * OUT_DIM + h * 512:
                                     e * OUT_DIM + (h + 1) * 512],
                            start=True, stop=True,
                        )
                    if rc % 2 == 0:
                        nc.vector.tensor_copy(out=xb[:, rc, :], in_=px[:])
                    else:
                        nc.scalar.copy(out=xb[:, rc, :], in_=px[:])
                for par in range(2):
                    cap = caps[e * 2 + par]
                    gcap = -(-cap // 128)      # in_ap groups = roundup128(cap)
                    nc.gpsimd.dma_scatter_add(
                        outs[par][:],
                        xb[:, par * gpar:par * gpar + gcap, :],
                        idx_sb[:, (e * 2 + par) * (pp // 16):
                               (e * 2 + par) * (pp // 16) + cap // 16],
                        cap,
                        cap,
                        OUT_DIM,
                        queue_num=par,
                    )

    nc.compile()
    return nc


def _get_program(pp, caps):
    key = (pp, caps)
    if key not in _cache:
        _cache[key] = _build_program(pp, caps)
    return _cache[key]


def _dedup_head(Yh, indh):
    """Pre-sum rows sharing a slot (exact: same W)."""
    order = np.argsort(indh, kind="stable")
    sind = indh[order]
    starts = np.concatenate(([0], np.nonzero(np.diff(sind))[0] + 1))
    sums = np.add.reduceat(Yh[order].astype(np.float32), starts, axis=0)
    return sums, sind[starts]


def kernel(Y, Ind, T, W):
    Y = np.asarray(Y, dtype=np.float32)
    Ind = np.asarray(Ind)
    W = np.asarray(W, dtype=np.float32)
    assert int(T) == T_SLOTS and Y.shape == (B, HEADS, K, HEAD_DIM)

    w_in = np.ascontiguousarray(
        W.transpose(1, 0, 2).reshape(HEAD_DIM, HEADS * OUT_DIM)
    ).astype(np.float16)

    # Host prep: dedup per head.
    dedup = [[_dedup_head(Y[b, e], Ind[b, e].astype(np.int64))
              for e in range(HEADS)] for b in range(B)]

    # Balanced 2-coloring of slots (instead of plain parity): greedily
    # assign colors so every (core, head)'s slot set splits near-evenly.
    # Tightens the per-call num_idxs caps (max over cores) by ~4%.
    inc = np.zeros((T_SLOTS, B * HEADS), dtype=bool)
    for b in range(B):
        for e in range(HEADS):
            inc[dedup[b][e][1], b * HEADS + e] = True
    imb = np.zeros(B * HEADS)
    color = np.zeros(T_SLOTS, dtype=np.int64)
    tot = np.zeros(2, dtype=np.int64)
    half = T_SLOTS // 2
    for s in np.argsort(-inc.sum(axis=1), kind="stable"):
        row = inc[s]
        c0 = ((imb + row) ** 2).sum()
        c1 = ((imb - row) ** 2).sum()
        c = 0 if c0 <= c1 else 1
        if tot[c] >= half:            # fixed tensor shape: max half per color
            c = 1 - c
        color[s] = c
        tot[c] += 1
        imb = imb + row if c == 0 else imb - row
    rank = np.zeros(T_SLOTS, dtype=np.int64)
    slots_of = []
    for c in range(2):
        sc = np.nonzero(color == c)[0]
        rank[sc] = np.arange(len(sc))
        slots_of.append(sc)

    per_core = []
    cnts = np.zeros((B, HEADS * 2), dtype=np.int64)
    for b in range(B):
        heads = []
        for e in range(HEADS):
            sums, slots = dedup[b][e]
            parts = []
            for par in range(2):
                m = color[slots] == par
                parts.append((sums[m], rank[slots[m]].astype(np.int16)))
                cnts[b, e * 2 + par] = int(m.sum())
            heads.append(parts)
        per_core.append(heads)
    maxcnt = int(cnts.max())
    pp = max(PP, -(-maxcnt // 128) * 128)
    kh = 2 * pp
    # Per-call num_idxs cap: shared across cores (SPMD), tight per call.
    caps = tuple(int(-(-int(c) // 16) * 16) or 16
                 for c in cnts.max(axis=0))

    in_maps = []
    for b in range(B):
        ytb = np.zeros((HEAD_DIM, HEADS * kh), dtype=np.float16)
        idx_cols = []
        for e in range(HEADS):
            for par in range(2):
                rows, slots = per_core[b][e][par]
                cnt = len(slots)
                base = e * kh + par * pp
                ytb[:, base:base + cnt] = rows.T.astype(np.float16)
                idxp = np.full(pp, T_SLOTS // 2, dtype=np.int16)
                idxp[:cnt] = slots
                idx_cols.append(np.tile(idxp.reshape(pp // 16, 16).T, (8, 1)))
        idx_in = np.ascontiguousarray(np.concatenate(idx_cols, axis=1),
                                      dtype=np.int16)
        in_maps.append({"yt": ytb, "w": w_in, "idx": idx_in})

    nc = _get_program(pp, caps)

    # The first execution of a freshly compiled NEFF occasionally wedges a
    # core (NRT_EXEC_UNIT_UNRECOVERABLE); a retry on a fresh execute has
    # been observed to recover.
    last_exc = None
    for attempt in range(3):
        try:
            res = run_bass_kernel_spmd(
                nc, in_maps, core_ids=list(range(NCORES)),
                trace=os.environ.get("ES_TRACE", "0") == "1",
            )
            break
        except Exception as exc:  # noqa: BLE001 - device flake, retry
            last_exc = exc
            import time as _time
            _time.sleep(2.0)
    else:
        raise last_exc
    kernel.last_results = res
    out = np.empty((B, T_SLOTS, OUT_DIM), dtype=np.float32)
    for b in range(B):
        for c, nm in enumerate(("out_even", "out_odd")):
            sc = slots_of[c]
            out[b, sc] = res.results[b][nm][:len(sc)].astype(np.float32)
    return out


# revision 19
# speedup vs baseline: 1.0283x; 1.0078x over previous
"""ExpertScatter TRN2 kernel — direct DMA scatter-add, parity-split outputs.

reference semantics:
    X = einsum('bekj,eji->beki', Y, W)          # per-head projection
    out[b] = zeros([T, I]); out[b, Ind[b,e,k]] += X[b,e,k]

Strategy (data-parallel over batch, 1 batch per NeuronCore):
  Per head e, project Yt chunks against W[e] on the PE (fp16 operands,
  fp32 PSUM), copy each [128, 1024] X chunk to SBUF (fp16), and
  scatter-add the rows straight into HBM output accumulators with
  gpsimd.dma_scatter_add (out[idx_i] += x_row_i).

  Duplicate-index hazard: descriptors of ONE scatter call race on HBM
  read-modify-write (probed: duplicates closer than ~256 positions lose
  adds). Fix: the host pre-sums rows of the same (head, slot) — exact,
  since they share W[e] — so each call has unique indices. Cross-head
  duplicates are safe: calls targeting the same tensor are
  WAW-serialized by the tile framework (next call's transfer waits the
  previous DMA-completion semaphore).

  The WAW chain leaves a ~3us bubble between consecutive scatter calls
  (sem propagation + Q7 descriptor-gen + DGE trigger delay). To fill
  those bubbles, the output is split by slot parity into TWO separate
  DRAM tensors (out_even = slots 0,2,..., out_odd = slots 1,3,...) with
  independent WAW chains; their transfers interleave on the shared DMA
  engines, and the host re-interleaves rows at the end (free).

  Zero-payload pad tokens target a dedicated DUMP ROW (row T/2) so
  their RMWs can never race a real slot's add, and per-call num_idxs is
  capped at the data's max real count (shared across cores, SPMD-safe).

  Total per-core DMA: Y 4MB + W 4MB + zero-init 8MB + scatter ~30MB
  = ~46MB at ~360GB/s model bandwidth (~98% DMA-device utilization),
  vs ~80MB for the sort+stage+gather pipeline (286us -> 139us).
"""

import os

import numpy as np

import concourse.bacc as bacc
import concourse.mybir as mybir
import concourse.tile as tile
from concourse.bass_utils import run_bass_kernel_spmd

# Problem constants (hardcoded per harness contract).
B = 8
HEADS = 16
K = 1024
HEAD_DIM = 128
OUT_DIM = 1024
T_SLOTS = 4096
NCORES = 8

PP = 512                 # padded rows per (head, parity); 487 on seed-0 data

F32 = mybir.dt.float32
F16 = mybir.dt.float16
I16 = mybir.dt.int16

XBUFS = int(os.environ.get("ES_XBUFS", "3"))
PBUFS = int(os.environ.get("ES_PBUFS", "3"))
YBUFS = int(os.environ.get("ES_YBUFS", "4"))

_cache = {}


def _build_program(pp, caps):
    # Packed layout: call h (= head*2 + color) occupies token range
    # [offs[h], offs[h] + caps[h]) in yt/idx; no fixed per-call stride.
    offs = [0]
    for c in caps:
        offs.append(offs[-1] + c)
    S = offs[-1]
    ghead = [(-(-caps[2 * e] // 128), -(-caps[2 * e + 1] // 128))
             for e in range(HEADS)]
    ngmax = max(ge + go for ge, go in ghead)
    # SBUF pressure guard for the (never-seen) large-pp fallback path.
    xbufs = XBUFS if pp <= 512 else 2
    ybufs = YBUFS if pp <= 512 else 2
    nc = bacc.Bacc("TRN2", target_bir_lowering=False, debug=False,
                   num_devices=NCORES, num_swdge_queues=2,
                   dynamic_dma_scratch_size=131072)

    yt = nc.dram_tensor("yt", [HEAD_DIM, S], F16,
                        kind="ExternalInput").ap()
    w = nc.dram_tensor("w", [HEAD_DIM, HEADS * OUT_DIM], F16,
                       kind="ExternalInput").ap()
    idx = nc.dram_tensor("idx", [128, S // 16], I16,
                         kind="ExternalInput").ap()
    # One extra "dump" row per tensor: zero-payload pad tokens target it
    # so their HBM read-modify-writes can never race a real slot's add.
    outs = [nc.dram_tensor(nm, [T_SLOTS // 2 + 1, OUT_DIM], F16,
                           kind="ExternalOutput").ap()
            for nm in ("out_even", "out_odd")]

    with tile.TileContext(nc) as tc:
        with (
            tc.tile_pool(name="const", bufs=1) as cpool,
            tc.tile_pool(name="yhead", bufs=ybufs) as ypool,
            tc.tile_pool(name="xblk", bufs=xbufs) as xpool,
            tc.tile_pool(name="psum", bufs=PBUFS, space="PSUM") as pspool,
        ):
            z = cpool.tile([128, OUT_DIM], F16, tag="z")
            nc.vector.memset(z[:], 0.0)
            idx_sb = cpool.tile([128, S // 16], I16, tag="idx")
            nc.sync.dma_start(out=idx_sb[:], in_=idx[:])
            w_sb = cpool.tile([128, HEADS * OUT_DIM], F16, tag="w")

            def load_w(e):
                nc.sync.dma_start(
                    out=w_sb[:, e * OUT_DIM:(e + 1) * OUT_DIM],
                    in_=w[:, e * OUT_DIM:(e + 1) * OUT_DIM])

            yts = {}

            def load_yt(e):
                wh = caps[2 * e] + caps[2 * e + 1]
                yt_e = ypool.tile([128, 2 * pp], F16, tag="yt")
                nc.sync.dma_start(out=yt_e[:, :wh],
                                  in_=yt[:, offs[2 * e]:offs[2 * e] + wh])
                yts[e] = yt_e

            # Early loads so heads 0-3 compute overlaps the zero-init.
            for e in range(4):
                load_w(e)
                load_yt(e)
            # Zero-init the accumulators: even chain issued from SP (early),
            # odd chain from ACT; both ahead of their first scatters.
            for t in range(T_SLOTS // 256):
                nc.sync.dma_start(out=outs[0][t * 128:(t + 1) * 128, :],
                                  in_=z[:])
            for t in range(T_SLOTS // 256):
                nc.scalar.dma_start(out=outs[1][t * 128:(t + 1) * 128, :],
                                    in_=z[:])
            for e in range(4, HEADS):
                load_w(e)

            for e in range(HEADS):
                ge, go = ghead[e]
                xb = xpool.tile([128, ngmax, OUT_DIM], F16, tag="xb")
                if e not in yts:
                    load_yt(e)
                yt_e = yts[e]
                # chunk rc of color par reads yt cols [cbase + rc*128, ...)
                # (cbase = 0 or caps[2e], carrying the unaligned boundary).
                for par, gpar_, cbase in ((0, ge, 0), (1, go, caps[2 * e])):
                    for rc in range(gpar_):
                        px = pspool.tile([128, OUT_DIM], F32, tag="px")
                        for h in range(2):
                            nc.tensor.matmul(
                                out=px[:, h * 512:(h + 1) * 512],
                                lhsT=yt_e[:, cbase + rc * 128:
                                          cbase + (rc + 1) * 128],
                                rhs=w_sb[:, e * OUT_DIM + h * 512:
                                         e * OUT_DIM + (h + 1) * 512],
                                start=True, stop=True,
                            )
                        g = par * ge + rc
                        if g % 2 == 0:
                            nc.vector.tensor_copy(out=xb[:, g, :], in_=px[:])
                        else:
                            nc.scalar.copy(out=xb[:, g, :], in_=px[:])
                for par in range(2):
                    cap = caps[e * 2 + par]
                    g0 = 0 if par == 0 else ge
                    gcap = ge if par == 0 else go
                    nc.gpsimd.dma_scatter_add(
                        outs[par][:],
                        xb[:, g0:g0 + gcap, :],
                        idx_sb[:, offs[e * 2 + par] // 16:
                               offs[e * 2 + par] // 16 + cap // 16],
                        cap,
                        cap,
                        OUT_DIM,
                        queue_num=par,
                    )

    nc.compile()
    return nc


def _get_program(pp, caps):
    key = (pp, caps)
    if key not in _cache:
        _cache[key] = _build_program(pp, caps)
    return _cache[key]


def _dedup_head(Yh, indh):
    """Pre-sum rows sharing a slot (exact: same W)."""
    order = np.argsort(indh, kind="stable")
    sind = indh[order]
    starts = np.concatenate(([0], np.nonzero(np.diff(sind))[0] + 1))
    sums = np.add.reduceat(Yh[order].astype(np.float32), starts, axis=0)
    return sums, sind[starts]


def kernel(Y, Ind, T, W):
    Y = np.asarray(Y, dtype=np.float32)
    Ind = np.asarray(Ind)
    W = np.asarray(W, dtype=np.float32)
    assert int(T) == T_SLOTS and Y.shape == (B, HEADS, K, HEAD_DIM)

    w_in = np.ascontiguousarray(
        W.transpose(1, 0, 2).reshape(HEAD_DIM, HEADS * OUT_DIM)
    ).astype(np.float16)

    # Host prep: dedup per head.
    dedup = [[_dedup_head(Y[b, e], Ind[b, e].astype(np.int64))
              for e in range(HEADS)] for b in range(B)]

    # Balanced 2-coloring of slots (instead of plain parity): greedily
    # assign colors so every (core, head)'s slot set splits near-evenly.
    # Tightens the per-call num_idxs caps (max over cores) by ~4%.
    inc = np.zeros((T_SLOTS, B * HEADS), dtype=bool)
    for b in range(B):
        for e in range(HEADS):
            inc[dedup[b][e][1], b * HEADS + e] = True
    imb = np.zeros(B * HEADS)
    color = np.zeros(T_SLOTS, dtype=np.int64)
    tot = np.zeros(2, dtype=np.int64)
    half = T_SLOTS // 2
    for s in np.argsort(-inc.sum(axis=1), kind="stable"):
        row = inc[s]
        c0 = ((imb + row) ** 2).sum()
        c1 = ((imb - row) ** 2).sum()
        c = 0 if c0 <= c1 else 1
        if tot[c] >= half:            # fixed tensor shape: max half per color
            c = 1 - c
        color[s] = c
        tot[c] += 1
        imb = imb + row if c == 0 else imb - row
    rank = np.zeros(T_SLOTS, dtype=np.int64)
    slots_of = []
    for c in range(2):
        sc = np.nonzero(color == c)[0]
        rank[sc] = np.arange(len(sc))
        slots_of.append(sc)

    per_core = []
    cnts = np.zeros((B, HEADS * 2), dtype=np.int64)
    for b in range(B):
        heads = []
        for e in range(HEADS):
            sums, slots = dedup[b][e]
            parts = []
            for par in range(2):
                m = color[slots] == par
                parts.append((sums[m], rank[slots[m]].astype(np.int16)))
                cnts[b, e * 2 + par] = int(m.sum())
            heads.append(parts)
        per_core.append(heads)
    maxcnt = int(cnts.max())
    pp = max(PP, -(-maxcnt // 128) * 128)
    kh = 2 * pp
    # Per-call num_idxs cap: shared across cores (SPMD), tight per call.
    caps = tuple(int(-(-int(c) // 16) * 16) or 16
                 for c in cnts.max(axis=0))

    offs = np.concatenate(([0], np.cumsum(caps)))
    S = int(offs[-1])
    in_maps = []
    for b in range(B):
        ytb = np.zeros((HEAD_DIM, S), dtype=np.float16)
        idx_cols = []
        for e in range(HEADS):
            for par in range(2):
                rows, slots = per_core[b][e][par]
                cnt = len(slots)
                cap = caps[e * 2 + par]
                base = int(offs[e * 2 + par])
                ytb[:, base:base + cnt] = rows.T.astype(np.float16)
                idxp = np.full(cap, T_SLOTS // 2, dtype=np.int16)
                idxp[:cnt] = slots
                idx_cols.append(np.tile(idxp.reshape(cap // 16, 16).T, (8, 1)))
        idx_in = np.ascontiguousarray(np.concatenate(idx_cols, axis=1),
                                      dtype=np.int16)
        in_maps.append({"yt": ytb, "w": w_in, "idx": idx_in})

    nc = _get_program(pp, caps)

    # The first execution of a freshly compiled NEFF occasionally wedges a
    # core (NRT_EXEC_UNIT_UNRECOVERABLE); a retry on a fresh execute has
    # been observed to recover.
    last_exc = None
    for attempt in range(3):
        try:
            res = run_bass_kernel_spmd(
                nc, in_maps, core_ids=list(range(NCORES)),
                trace=os.environ.get("ES_TRACE", "0") == "1",
            )
            break
        except Exception as exc:  # noqa: BLE001 - device flake, retry
            last_exc = exc
            import time as _time
            _time.sleep(2.0)
    else:
        raise last_exc
    kernel.last_results = res
    out = np.empty((B, T_SLOTS, OUT_DIM), dtype=np.float32)
    for b in range(B):
        for c, nm in enumerate(("out_even", "out_odd")):
            sc = slots_of[c]
            out[b, sc] = res.results[b][nm][:len(sc)].astype(np.float32)
    return out
